# revision 1
# baseline (speedup 1.0000x reference)
"""AdaptiveDCA Trainium2 kernel: 4-branch dilated 3x3 attention with gated concat.

Sharding: data-parallel over batch B=8 across 8 NeuronCores (1 image/core).
Per-core layout (Option B): channels on partitions, flattened HW on free axis.
  - qkv projection: TensorE GEMM, w^T stationary, x streaming (f32).
  - K/V stored zero-padded (128, 80*80) so every 3x3 dilated tap is a strided
    in-bounds slice; OOB taps give logit 0 / value 0 == reference semantics.
  - logits: DVE q*k_shift mul (bf16) + TensorE ones-matmul partition-reduce.
  - softmax: ScalarE exp (no max-sub needed; logits ~N(0,1)), TensorE denom
    reduce, DVE reciprocal.
  - AV: DVE E_row*v_shift muls + TensorE identity-matmul PSUM accumulation.
  - gate: folded into V evacuation scale (per-branch scalar).
"""

import os
import sys

sys.path.insert(0, "/opt/trn_rl_repo")

import numpy as np
import ml_dtypes

import concourse.bass as bass
import concourse.tile as tile
from concourse import bacc, mybir
from concourse.bass_utils import run_bass_kernel_spmd

F32 = mybir.dt.float32
BF16 = mybir.dt.bfloat16
AF = mybir.ActivationFunctionType

P = 128
N = 4096          # 64*64
H = W = 64
PAD = 8
WP = 80           # padded width/height
NPAD = WP * WP    # 6400
DILS = (1, 2, 4, 8)
NQ = 512          # slice of N processed per attention step (1-bank PSUM tiles)
QROWS = NQ // W   # image rows per attention slice
TAPS = [(di, dj) for di in (-1, 0, 1) for dj in (-1, 0, 1)]

_CACHE = {}


def _dma_engines(nc):
    return [nc.sync, nc.scalar, nc.gpsimd]


def _sub(ap, off_elems, dims):
    """Manual free-dim sub-AP of a 2D tile AP (keeps partition dim)."""
    part = list(ap.ap[0])
    return bass.AP(
        tensor=ap.tensor,
        offset=ap.offset + off_elems * 1,
        ap=[part] + [[s, n] for s, n in dims],
    )


def _build():
    nc = bacc.Bacc("TRN2", target_bir_lowering=False, debug=False, num_devices=8)

    x_d = nc.dram_tensor("x", [512, N], BF16, kind="ExternalInput").ap()
    wq_d = nc.dram_tensor("wqkvT", [512, 1536], BF16, kind="ExternalInput").ap()
    wg_d = nc.dram_tensor("wgT", [512, 4], BF16, kind="ExternalInput").ap()
    bg_d = nc.dram_tensor("bg", [4, 1], F32, kind="ExternalInput").ap()
    bd_d = nc.dram_tensor("blkdiag", [128, 128], BF16, kind="ExternalInput").ap()
    s2_d = nc.dram_tensor("sel2", [128, 2], BF16, kind="ExternalInput").ap()
    id_d = nc.dram_tensor("ident", [128, 128], BF16, kind="ExternalInput").ap()
    out_d = nc.dram_tensor("out", [512, N], F32, kind="ExternalOutput").ap()

    from contextlib import ExitStack

    with tile.TileContext(nc) as tc, ExitStack() as ctx:
        consts = ctx.enter_context(tc.tile_pool(name="consts", bufs=1))
        xpool = ctx.enter_context(tc.tile_pool(name="xp", bufs=1))
        qpool = ctx.enter_context(tc.tile_pool(name="qp", bufs=2))
        kpool = ctx.enter_context(tc.tile_pool(name="kp", bufs=2))
        vpool = ctx.enter_context(tc.tile_pool(name="vp", bufs=2))
        gps = ctx.enter_context(tc.tile_pool(name="gps", bufs=4, space="PSUM"))
        gemmps = ctx.enter_context(tc.tile_pool(name="gemmps", bufs=2, space="PSUM"))
        epool = ctx.enter_context(tc.tile_pool(name="ep", bufs=2))
        prodp = ctx.enter_context(tc.tile_pool(name="prodp", bufs=5))
        pvp = ctx.enter_context(tc.tile_pool(name="pvp", bufs=5))
        rpool = ctx.enter_context(tc.tile_pool(name="rp", bufs=2))
        rbpool = ctx.enter_context(tc.tile_pool(name="rbp", bufs=2))
        dsbpool = ctx.enter_context(tc.tile_pool(name="dsbp", bufs=2))
        opool = ctx.enter_context(tc.tile_pool(name="op", bufs=3))
        gate_pool = ctx.enter_context(tc.tile_pool(name="gatep", bufs=1))

        # ---- load constants / inputs to SBUF ----
        w_sb = consts.tile([P, 4, 1536], BF16)
        for ct in range(4):
            _dma_engines(nc)[ct % 2].dma_start(w_sb[:, ct, :], wq_d[ct * P:(ct + 1) * P, :])
        wg_sb = consts.tile([P, 4, 4], BF16)
        for ct in range(4):
            nc.sync.dma_start(wg_sb[:, ct, :], wg_d[ct * P:(ct + 1) * P, :])
        bd_sb = consts.tile([P, P], BF16)
        nc.sync.dma_start(bd_sb[:], bd_d[:])
        s2_sb = consts.tile([P, 2], BF16)
        nc.sync.dma_start(s2_sb[:], s2_d[:])
        id_sb = consts.tile([P, P], BF16)
        nc.sync.dma_start(id_sb[:], id_d[:])
        bg_sb = gate_pool.tile([4, 1], F32)
        nc.sync.dma_start(bg_sb[:], bg_d[:])

        engs0 = _dma_engines(nc)
        x_sb = []
        for ct in range(4):
            xt = xpool.tile([P, N], BF16, tag=f"x{ct}", name=f"x{ct}")
            x_sb.append(xt)
        qn = N // 4
        for qtr in range(4):
            for ct in range(4):
                engs0[(qtr + ct) % 2].dma_start(
                    x_sb[ct][:, qtr * qn:(qtr + 1) * qn],
                    x_d[ct * P:(ct + 1) * P, qtr * qn:(qtr + 1) * qn])

        # ---- gate: logits = (sum_n w_g^T x)/N + b_gate ; softmax over 4 ----
        gl_parts = gate_pool.tile([4, 8], F32)
        trash = gate_pool.tile([4, 512], F32)
        for ch in range(8):
            ps4 = gemmps.tile([4, 512], F32, tag="g")
            for ct in range(4):
                nc.tensor.matmul(ps4[:], wg_sb[:, ct, :], x_sb[ct][:, ch * 512:(ch + 1) * 512],
                                 start=(ct == 0), stop=(ct == 3))
            nc.scalar.activation(trash[:], ps4[:], AF.Copy,
                                 accum_out=gl_parts[:, ch:ch + 1])
        glog4 = gate_pool.tile([4, 1], F32)
        nc.vector.reduce_sum(glog4[:], gl_parts[:], axis=mybir.AxisListType.X)
        logit4 = gate_pool.tile([4, 1], F32)
        nc.vector.scalar_tensor_tensor(logit4[:], glog4[:], 1.0 / N, bg_sb[:],
                                       op0=mybir.AluOpType.mult,
                                       op1=mybir.AluOpType.add)
        gexp4 = gate_pool.tile([4, 1], F32)
        nc.scalar.activation(gexp4[:], logit4[:], AF.Exp)
        gexpT = gate_pool.tile([1, 4], F32)
        nc.sync.dma_start(gexpT[:], gexp4[:])
        gsum = gate_pool.tile([1, 1], F32)
        nc.vector.reduce_sum(gsum[:], gexpT[:], axis=mybir.AxisListType.X)
        grec = gate_pool.tile([1, 1], F32)
        nc.vector.reciprocal_approx_fast(grec[:], gsum[:])
        gates = gate_pool.tile([1, 4], F32)
        nc.vector.tensor_scalar_mul(gates[:], gexpT[:], grec[:])
        gtmp = nc.dram_tensor("gtmp", [1, 4], F32).ap()
        nc.sync.dma_start(gtmp[:], gates[:])
        gate_bc = []
        for b in range(4):
            gb = gate_pool.tile([P, 1], F32, tag=f"gbc{b}")
            src = gtmp[0:1, b:b + 1]
            bc = bass.AP(tensor=src.tensor, offset=src.offset,
                         ap=[[0, P], [1, 1]])
            nc.sync.dma_start(gb[:], bc)
            gate_bc.append(gb)

        # ---- per-branch GEMM + attention ----
        for b, dil in enumerate(DILS):
            Q = qpool.tile([P, N], BF16)
            Kp = kpool.tile([P, NPAD], BF16)
            Vp = vpool.tile([P, NPAD], BF16)
            if b == 0:
                nc.vector.memset(Kp[:], 0.0)
                nc.vector.memset(Vp[:], 0.0)
            else:
                nc.gpsimd.memset(Kp[:], 0.0)
                nc.gpsimd.memset(Vp[:], 0.0)

            for kind, obase in (("q", b * P), ("k", 512 + b * P), ("v", 1024 + b * P)):
                for ch in range(4):
                    ps = gemmps.tile([P, 1024], F32, tag="g")
                    for ct in range(4):
                        for hf in range(2):
                            nc.tensor.matmul(
                                ps[:, hf * 512:(hf + 1) * 512],
                                w_sb[:, ct, obase:obase + P],
                                x_sb[ct][:, ch * 1024 + hf * 512:
                                          ch * 1024 + (hf + 1) * 512],
                                start=(ct == 0), stop=(ct == 3))
                    ps_v = ps[:].rearrange("p (r c) -> p r c", c=W)
                    if kind == "q":
                        nc.scalar.activation(
                            Q[:, ch * 1024:(ch + 1) * 1024], ps[:], AF.Copy)
                    else:
                        dst_t = Kp if kind == "k" else Vp
                        dst = _sub(dst_t[:], (PAD + ch * 16) * WP + PAD,
                                   [[WP, 16], [1, W]])
                        if kind == "k":
                            nc.scalar.activation(dst, ps_v, AF.Copy)
                        else:
                            nc.scalar.activation(dst, ps_v, AF.Copy,
                                                 scale=gate_bc[b][:])

            Q3 = Q[:].rearrange("p (r c) -> p r c", c=W)
            for qt in range(N // NQ):
                r0 = qt * QROWS
                engs = _dma_engines(nc)
                prods = []
                for t, (di, dj) in enumerate(TAPS):
                    off = (PAD + r0 + di * dil) * WP + (PAD + dj * dil)
                    prod = prodp.tile([P, NQ], BF16, tag="prod", name=f"prod_{b}_{qt}_{t}")
                    kv = _sub(Kp[:], off, [[WP, QROWS], [1, W]])
                    pv_out = prod[:].rearrange("p (r c) -> p r c", c=W)
                    nc.vector.tensor_mul(pv_out, Q3[:, r0:r0 + QROWS, :], kv)
                    prods.append(prod)
                Edall = epool.tile([P, 9, NQ], BF16, tag="Edall", name=f"Edall_{b}_{qt}")
                for t in range(9):
                    Ld = gps.tile([P, NQ], F32, tag="ps", name=f"Ld_{b}_{qt}_{t}")
                    nc.tensor.matmul(Ld[:], bd_sb[:], prods[t][:],
                                     start=True, stop=True)
                    nc.scalar.activation(Edall[:, t, :], Ld[:], AF.Exp, scale=0.125)
                # --- denominator: sel2 row-extract matmuls (single group) ---
                D = gps.tile([2, NQ], F32, tag="ps", name=f"D_{b}_{qt}")
                for t in range(9):
                    nc.tensor.matmul(D[:], s2_sb[:],
                                     _sub(Edall[:], t * NQ, [[1, NQ]]),
                                     start=(t == 0), stop=(t == 8))
                Dsb = dsbpool.tile([2, NQ], F32, tag="dsb", name=f"Dsb_{b}_{qt}")
                nc.scalar.activation(Dsb[:], D[:], AF.Copy)
                R = rpool.tile([2, NQ], F32, tag="r", name=f"R_{b}_{qt}")
                nc.vector.reciprocal_approx_fast(R[:], Dsb[:])
                rtmp = nc.dram_tensor(f"rtmp_{b}_{qt}", [2, NQ], F32).ap()
                engs[qt % 3].dma_start(rtmp[:], R[:])
                Rb = rbpool.tile([P, NQ], F32, tag="rb", name=f"Rb_{b}_{qt}")
                bc = bass.AP(tensor=rtmp.tensor, offset=rtmp.offset,
                             ap=[[NQ, 2], [0, 64], [1, NQ]])
                engs[(qt + 1) % 3].dma_start(Rb[:], bc)
                # --- AV: weighted-V products accumulated via identity matmul ---
                AV = gps.tile([P, NQ], F32, tag="ps", name=f"AV_{b}_{qt}")
                for t, (di, dj) in enumerate(TAPS):
                    off = (PAD + r0 + di * dil) * WP + (PAD + dj * dil)
                    pv = pvp.tile([P, NQ], BF16, tag="pv", name=f"pv_{b}_{qt}_{t}")
                    vv = _sub(Vp[:], off, [[WP, QROWS], [1, W]])
                    pvt = pv[:].rearrange("p (r c) -> p r c", c=W)
                    ev = _sub(Edall[:], t * NQ, [[W, QROWS], [1, W]])
                    nc.vector.tensor_mul(pvt, ev, vv)
                    nc.tensor.matmul(AV[:], id_sb[:], pv[:],
                                     start=(t == 0), stop=(t == 8))
                # --- normalize + store ---
                osb = opool.tile([P, NQ], F32, tag="osb", name=f"osb_{b}_{qt}")
                nc.vector.tensor_mul(osb[:], AV[:], Rb[:])
                engs[(qt + 2) % 3].dma_start(
                    out_d[b * P:(b + 1) * P, qt * NQ:(qt + 1) * NQ], osb[:])

    nc.compile()
    return nc


def _consts():
    bf = ml_dtypes.bfloat16
    blkdiag = np.zeros((128, 128), np.float32)
    blkdiag[:64, :64] = 1.0
    blkdiag[64:, 64:] = 1.0
    sel2 = np.zeros((128, 2), np.float32)
    sel2[0, 0] = 1.0
    sel2[64, 1] = 1.0
    ident = np.eye(128, dtype=np.float32)
    return (blkdiag.astype(bf), sel2.astype(bf), ident.astype(bf))


def kernel(x, w_qkv, w_gate, b_gate):
    # harness may pass jax arrays (setup_inputs returns them); coerce on host
    x = np.asarray(x, dtype=np.float32)
    w_qkv = np.asarray(w_qkv, dtype=np.float32)
    w_gate = np.asarray(w_gate, dtype=np.float32)
    b_gate = np.asarray(b_gate, dtype=np.float32)
    if "nc" not in _CACHE:
        _CACHE["nc"] = _build()
    nc = _CACHE["nc"]
    blkdiag, sel2, ident = _consts()
    wqkvT = np.ascontiguousarray(w_qkv.T).astype(ml_dtypes.bfloat16)
    wgT = np.ascontiguousarray(w_gate.T).astype(ml_dtypes.bfloat16)
    bg = b_gate.reshape(4, 1).astype(np.float32)
    in_maps = []
    for b in range(8):
        in_maps.append({
            "x": np.ascontiguousarray(x[b].reshape(512, N)).astype(ml_dtypes.bfloat16),
            "wqkvT": wqkvT, "wgT": wgT, "bg": bg,
            "blkdiag": blkdiag, "sel2": sel2, "ident": ident,
        })
    res = run_bass_kernel_spmd(nc, in_maps, core_ids=list(range(8)),
                               trace=bool(int(os.environ.get("KTRACE", "0"))))
    _CACHE["last"] = res
    out = np.stack([np.asarray(res.results[b]["out"], dtype=np.float32)
                    .reshape(512, H, W) for b in range(8)])
    return out



# revision 9
# speedup vs baseline: 1.0426x; 1.0426x over previous
"""AdaptiveDCA Trainium2 kernel: 4-branch dilated 3x3 attention with gated concat.

Sharding: data-parallel over batch B=8 across 8 NeuronCores (1 image/core).
Per-core layout: channels on partitions, flattened HW on free axis.
  - qkv projection: TensorE GEMM, w^T stationary, x streaming (bf16).
  - K/V stored zero-padded (128, 80*80) so every 3x3 dilated tap is a strided
    in-bounds slice; OOB taps give logit 0 / value 0 == reference semantics.
  - logits: single fused DVE q*k_shift mul (5-dim AP covering all 9 taps)
    + TensorE block-diag ones matmul partition-reduce (logits broadcast 64x).
  - softmax: ScalarE exp on tap-pairs (PSUM pair tiles), denominator via a
    compact side-channel: DMA-gather partitions {0,64} of E into [18, NQ]
    (2 heads x 9 taps on partitions), then ONE TensorE matmul with an
    [18, 128] head-map stationary gives the denominator broadcast to all
    128 channels; DVE reciprocal directly yields Rb [128, NQ].
  - AV: fused DVE E*v_shift mul + TensorE identity-matmul PSUM accumulation.
  - gate + normalize fused in one DVE scalar_tensor_tensor:
    out = (AV * gate_b) * Rb.
  - GpSimd (Pool) takes the AV-mul of branch 3 and K/V evac of branch 3 to
    offload DVE/ScalarE.
"""

import os
import sys

sys.path.insert(0, "/opt/trn_rl_repo")

import numpy as np
import ml_dtypes

import concourse.bass as bass
import concourse.tile as tile
from concourse import bacc, mybir
from concourse.bass_utils import run_bass_kernel_spmd

F32 = mybir.dt.float32
BF16 = mybir.dt.bfloat16
AF = mybir.ActivationFunctionType
ALU = mybir.AluOpType

P = 128
N = 4096          # 64*64
H = W = 64
PAD = 8
WP = 80           # padded width/height
NPAD = WP * WP    # 6400
DILS = (1, 2, 4, 8)
NQ = 512          # positions per attention step (8 image rows)
QROWS = NQ // W   # image rows per attention slice
NSTEP = N // NQ   # 8 steps per branch
EPITCH = 9 * NQ   # Edall per-partition elements

# which branches run the AV product on GpSimd / K,V evac tweaks
AV_POOL_BRANCHES = ()
QK_POOL_BRANCHES = ()

_CACHE = {}


def _sub(ap, off_elems, dims):
    """Manual free-dim sub-AP of a tile AP (keeps partition dim)."""
    part = list(ap.ap[0])
    return bass.AP(
        tensor=ap.tensor,
        offset=ap.offset + off_elems,
        ap=[part] + [[s, n] for s, n in dims],
    )


def _build():
    nc = bacc.Bacc("TRN2", target_bir_lowering=False, debug=False, num_devices=8)

    x_d = nc.dram_tensor("x", [512, N], BF16, kind="ExternalInput").ap()
    wq_d = nc.dram_tensor("wqkvT", [512, 1536], BF16, kind="ExternalInput").ap()
    wg_d = nc.dram_tensor("wgT", [512, 4], BF16, kind="ExternalInput").ap()
    bg_d = nc.dram_tensor("bg", [4, 1], F32, kind="ExternalInput").ap()
    bd_d = nc.dram_tensor("blkdiag", [128, 128], BF16, kind="ExternalInput").ap()
    sh_d = nc.dram_tensor("selhead", [18, 128], BF16, kind="ExternalInput").ap()
    id_d = nc.dram_tensor("ident", [128, 128], BF16, kind="ExternalInput").ap()
    out_d = nc.dram_tensor("out", [512, N], F32, kind="ExternalOutput").ap()

    from contextlib import ExitStack

    with tile.TileContext(nc) as tc, ExitStack() as ctx:
        consts = ctx.enter_context(tc.tile_pool(name="consts", bufs=1))
        xpool = ctx.enter_context(tc.tile_pool(name="xp", bufs=1))
        qpool = ctx.enter_context(tc.tile_pool(name="qp", bufs=2))
        kpool = ctx.enter_context(tc.tile_pool(name="kp", bufs=2))
        vpool = ctx.enter_context(tc.tile_pool(name="vp", bufs=2))
        # PSUM: pair-tiles (2 banks) for logits, 1-bank tiles for gemm/AV/D
        ldps = ctx.enter_context(tc.tile_pool(name="ldps", bufs=2, space="PSUM"))
        sps = ctx.enter_context(tc.tile_pool(name="sps", bufs=4, space="PSUM"))
        epool = ctx.enter_context(tc.tile_pool(name="ep", bufs=2))
        ecpool = ctx.enter_context(tc.tile_pool(name="ecp", bufs=1))
        prodp = ctx.enter_context(tc.tile_pool(name="prodp", bufs=2))
        pvp = ctx.enter_context(tc.tile_pool(name="pvp", bufs=1))
        rbpool = ctx.enter_context(tc.tile_pool(name="rbp", bufs=1))
        opool = ctx.enter_context(tc.tile_pool(name="op", bufs=2))
        gate_pool = ctx.enter_context(tc.tile_pool(name="gatep", bufs=1))

        # ---- load constants / inputs to SBUF ----
        w_sb = consts.tile([P, 4, 1536], BF16)
        for ct in range(4):
            nc.sync.dma_start(w_sb[:, ct, :], wq_d[ct * P:(ct + 1) * P, :])
        wg_sb = consts.tile([P, 4, 4], BF16)
        for ct in range(4):
            nc.sync.dma_start(wg_sb[:, ct, :], wg_d[ct * P:(ct + 1) * P, :])
        bd_sb = consts.tile([P, P], BF16)
        nc.sync.dma_start(bd_sb[:], bd_d[:])
        sh_sb = consts.tile([18, P], BF16)
        nc.sync.dma_start(sh_sb[:], sh_d[:])
        id_sb = consts.tile([P, P], BF16)
        nc.sync.dma_start(id_sb[:], id_d[:])
        bg_sb = gate_pool.tile([4, 1], F32)
        nc.sync.dma_start(bg_sb[:], bg_d[:])

        x_sb = []
        for ct in range(4):
            xt = xpool.tile([P, N], BF16, tag=f"x{ct}", name=f"x{ct}")
            x_sb.append(xt)
        qn = N // 4
        for qtr in range(4):
            for ct in range(4):
                nc.sync.dma_start(
                    x_sb[ct][:, qtr * qn:(qtr + 1) * qn],
                    x_d[ct * P:(ct + 1) * P, qtr * qn:(qtr + 1) * qn])

        # ---- gate: logits = (sum_n w_g^T x)/N + b_gate ; softmax over 4 ----
        gl_parts = gate_pool.tile([4, 8], F32)
        trash = gate_pool.tile([4, 512], F32)
        for ch in range(8):
            ps4 = sps.tile([4, 512], F32, tag="s", name=f"gps{ch}")
            for ct in range(4):
                nc.tensor.matmul(ps4[:], wg_sb[:, ct, :],
                                 x_sb[ct][:, ch * 512:(ch + 1) * 512],
                                 start=(ct == 0), stop=(ct == 3))
            nc.scalar.activation(trash[:], ps4[:], AF.Copy,
                                 accum_out=gl_parts[:, ch:ch + 1])
        glog4 = gate_pool.tile([4, 1], F32)
        nc.vector.reduce_sum(glog4[:], gl_parts[:], axis=mybir.AxisListType.X)
        logit4 = gate_pool.tile([4, 1], F32)
        nc.vector.scalar_tensor_tensor(logit4[:], glog4[:], 1.0 / N, bg_sb[:],
                                       op0=ALU.mult, op1=ALU.add)
        gexp4 = gate_pool.tile([4, 1], F32)
        nc.scalar.activation(gexp4[:], logit4[:], AF.Exp)
        gexpT = gate_pool.tile([1, 4], F32)
        nc.sync.dma_start(gexpT[:], gexp4[:])
        gsum = gate_pool.tile([1, 1], F32)
        nc.vector.reduce_sum(gsum[:], gexpT[:], axis=mybir.AxisListType.X)
        grec = gate_pool.tile([1, 1], F32)
        nc.vector.reciprocal_approx_fast(grec[:], gsum[:])
        gates = gate_pool.tile([1, 4], F32)
        nc.vector.tensor_scalar_mul(gates[:], gexpT[:], grec[:])
        gtmp = nc.dram_tensor("gtmp", [1, 4], F32).ap()
        nc.sync.dma_start(gtmp[:], gates[:])
        gate_bc = []
        for b in range(4):
            gb = gate_pool.tile([P, 1], F32, tag=f"gbc{b}")
            src = gtmp[0:1, b:b + 1]
            bc = bass.AP(tensor=src.tensor, offset=src.offset,
                         ap=[[0, P], [1, 1]])
            nc.sync.dma_start(gb[:], bc)
            gate_bc.append(gb)

        # tap-pair grouping for batched exp: (0,1) (2,3) (4,5) (6,7) (8,)
        PAIRS = [(0, 1), (2, 3), (4, 5), (6, 7), (8,)]

        def emit_prod(b, dil, s, Q, Kp):
            """Stage A: fused QK products for step s (one step ahead)."""
            r0 = s * QROWS
            prod = prodp.tile([P, 9, NQ], BF16, tag="prod",
                              name=f"prod_{b}_{s}")
            for di in range(3):
                eng = nc.gpsimd if (b, di) in QK_POOL_BRANCHES else nc.vector
                koff = (PAD + r0 + (di - 1) * dil) * WP + (PAD - dil)
                in1 = _sub(Kp[:], koff, [[dil, 3], [WP, QROWS], [1, W]])
                in0 = _sub(Q[:], r0 * W, [[0, 3], [W, QROWS], [1, W]])
                pout = _sub(prod[:], 3 * di * NQ, [[NQ, 3], [W, QROWS], [1, W]])
                eng.tensor_mul(pout, in0, in1)
            return prod

        def emit_back(st):
            """Stage C: denominator, AV, normalize, store for a finished step."""
            if st is None:
                return
            b, dil, s, Edall, Ec, Vp = st
            r0 = s * QROWS
            D = sps.tile([P, NQ], F32, tag="s", name=f"D_{b}_{s}")
            nc.tensor.matmul(D[:], sh_sb[:], Ec[:], start=True, stop=True)
            pv = pvp.tile([P, 9, NQ], BF16, tag="pv", name=f"pv_{b}_{s}")
            AV = sps.tile([P, NQ], F32, tag="s", name=f"AV_{b}_{s}")
            for di in range(3):
                eng = nc.gpsimd if (b, di) in AV_POOL_BRANCHES else nc.vector
                voff = (PAD + r0 + (di - 1) * dil) * WP + (PAD - dil)
                vin1 = _sub(Vp[:], voff, [[dil, 3], [WP, QROWS], [1, W]])
                ein0 = _sub(Edall[:], 3 * di * NQ, [[NQ, 3], [W, QROWS], [1, W]])
                pvout = _sub(pv[:], 3 * di * NQ, [[NQ, 3], [W, QROWS], [1, W]])
                eng.tensor_mul(pvout, ein0, vin1)
                for t in range(3 * di, 3 * di + 3):
                    nc.tensor.matmul(AV[:], id_sb[:],
                                     _sub(pv[:], t * NQ, [[1, NQ]]),
                                     start=(t == 0), stop=(t == 8))
            Rb = rbpool.tile([P, NQ], F32, tag="rb", name=f"Rb_{b}_{s}")
            nc.vector.reciprocal_approx_fast(Rb[:], D[:])
            osb = opool.tile([P, NQ], F32, tag="osb", name=f"osb_{b}_{s}")
            nc.vector.scalar_tensor_tensor(osb[:], AV[:], gate_bc[b][:],
                                           Rb[:], op0=ALU.mult, op1=ALU.mult)
            (nc.sync if s % 2 == 0 else nc.gpsimd).dma_start(
                out_d[b * P:(b + 1) * P, s * NQ:(s + 1) * NQ], osb[:])

        # ---- per-branch GEMM + 3-stage software-pipelined attention ----
        prev = None          # pending stage-C state
        prev_prod = None     # prod tile for the branch's next front step
        for b, dil in enumerate(DILS):
            Q = qpool.tile([P, N], BF16, tag="Q", name=f"Q{b}")
            Kp = kpool.tile([P, NPAD], BF16, tag=f"Kp{b % 2}", bufs=1,
                            name=f"Kp{b}")
            Vp = vpool.tile([P, NPAD], BF16, tag=f"Vp{b % 2}", bufs=1,
                            name=f"Vp{b}")
            if b < 2:
                # zero borders once; interiors are fully overwritten each use
                nc.gpsimd.memset(Kp[:], 0.0)
                nc.gpsimd.memset(Vp[:], 0.0)

            # GEMM: 512-col chunks; kinds ordered K, Q, V
            for kind, obase in (("k", 512 + b * P), ("q", b * P), ("v", 1024 + b * P)):
                for ch in range(8):
                    ps = sps.tile([P, 512], F32, tag="s", name=f"g_{b}_{kind}_{ch}")
                    for ct in range(4):
                        nc.tensor.matmul(
                            ps[:], w_sb[:, ct, obase:obase + P],
                            x_sb[ct][:, ch * 512:(ch + 1) * 512],
                            start=(ct == 0), stop=(ct == 3))
                    if kind == "q":
                        nc.scalar.activation(Q[:, ch * 512:(ch + 1) * 512],
                                             ps[:], AF.Copy)
                    else:
                        dst_t = Kp if kind == "k" else Vp
                        dst = _sub(dst_t[:], (PAD + ch * QROWS) * WP + PAD,
                                   [[WP, QROWS], [1, W]])
                        ps_v = ps[:].rearrange("p (r c) -> p r c", c=W)
                        nc.scalar.activation(dst, ps_v, AF.Copy)

            prev_prod = emit_prod(b, dil, 0, Q, Kp)

            for s in range(NSTEP):
                prod = prev_prod
                # ---- stage B part 1: logit pairs 0,1 + exp ----
                Edall = epool.tile([P, 9, NQ], BF16, tag="Edall",
                                   name=f"Edall_{b}_{s}")
                for pi, pair in enumerate(PAIRS):
                    npair = len(pair)
                    Ld = ldps.tile([P, npair, NQ], F32, tag="ld",
                                   name=f"Ld_{b}_{s}_{pi}")
                    for j, t in enumerate(pair):
                        nc.tensor.matmul(Ld[:, j, :], bd_sb[:],
                                         _sub(prod[:], t * NQ, [[1, NQ]]),
                                         start=True, stop=True)
                    nc.scalar.activation(Edall[:, pair[0]:pair[0] + npair, :],
                                         Ld[:], AF.Exp, scale=0.125)
                    if pi == 1:
                        # stage C of previous step fills the PE gap while
                        # ScalarE chews through the exp pairs
                        emit_back(prev)
                        # stage A: products for next step (keeps DVE ahead)
                        if s + 1 < NSTEP:
                            prev_prod = emit_prod(b, dil, s + 1, Q, Kp)

                # ---- compact denominator gather ----
                Ec = ecpool.tile([18, NQ], BF16, tag="ec", name=f"Ec_{b}_{s}")
                esrc = bass.AP(tensor=Edall.tensor, offset=Edall[:].offset,
                               ap=[[64 * EPITCH, 2], [NQ, 9], [1, NQ]])
                nc.sync.dma_start(Ec[:], esrc)
                prev = (b, dil, s, Edall, Ec, Vp)

        emit_back(prev)

    nc.compile()
    return nc


def _consts():
    bf = ml_dtypes.bfloat16
    blkdiag = np.zeros((128, 128), np.float32)
    blkdiag[:64, :64] = 1.0
    blkdiag[64:, 64:] = 1.0
    selhead = np.zeros((18, 128), np.float32)
    selhead[:9, :64] = 1.0
    selhead[9:, 64:] = 1.0
    ident = np.eye(128, dtype=np.float32)
    return (blkdiag.astype(bf), selhead.astype(bf), ident.astype(bf))


def kernel(x, w_qkv, w_gate, b_gate):
    x = np.asarray(x, dtype=np.float32)
    w_qkv = np.asarray(w_qkv, dtype=np.float32)
    w_gate = np.asarray(w_gate, dtype=np.float32)
    b_gate = np.asarray(b_gate, dtype=np.float32)
    if "nc" not in _CACHE:
        _CACHE["nc"] = _build()
    nc = _CACHE["nc"]
    blkdiag, selhead, ident = _consts()
    wqkvT = np.ascontiguousarray(w_qkv.T).astype(ml_dtypes.bfloat16)
    wgT = np.ascontiguousarray(w_gate.T).astype(ml_dtypes.bfloat16)
    bg = b_gate.reshape(4, 1).astype(np.float32)
    in_maps = []
    for b in range(8):
        in_maps.append({
            "x": np.ascontiguousarray(x[b].reshape(512, N)).astype(ml_dtypes.bfloat16),
            "wqkvT": wqkvT, "wgT": wgT, "bg": bg,
            "blkdiag": blkdiag, "selhead": selhead, "ident": ident,
        })
    res = run_bass_kernel_spmd(nc, in_maps, core_ids=list(range(8)),
                               trace=bool(int(os.environ.get("KTRACE", "0"))))
    _CACHE["last"] = res
    out = np.stack([np.asarray(res.results[b]["out"], dtype=np.float32)
                    .reshape(512, H, W) for b in range(8)])
    return out


# revision 12
# speedup vs baseline: 1.3043x; 1.2510x over previous
"""AdaptiveDCA Trainium2 kernel: 4-branch dilated 3x3 attention with gated concat.

Sharding: data-parallel over batch B=8 across 8 NeuronCores (1 image/core).
Per-core layout: channels on partitions, flattened HW on free axis.
  - qkv projection: TensorE GEMM, w^T stationary, x streaming (bf16).
  - K/V stored zero-padded (128, 80*80) so every 3x3 dilated tap is a strided
    in-bounds slice; OOB taps give logit 0 / value 0 == reference semantics.
  - logits: single fused DVE q*k_shift mul (5-dim AP covering all 9 taps)
    + TensorE block-diag ones matmul partition-reduce (logits broadcast 64x).
  - softmax: ScalarE exp on tap-pairs (PSUM pair tiles), denominator via a
    compact side-channel: DMA-gather partitions {0,64} of E into [18, NQ]
    (2 heads x 9 taps on partitions), then ONE TensorE matmul with an
    [18, 128] head-map stationary gives the denominator broadcast to all
    128 channels; DVE reciprocal directly yields Rb [128, NQ].
  - AV: fused DVE E*v_shift mul + TensorE identity-matmul PSUM accumulation.
  - gate + normalize fused in one DVE scalar_tensor_tensor:
    out = (AV * gate_b) * Rb.
  - GpSimd (Pool) takes the AV-mul of branch 3 and K/V evac of branch 3 to
    offload DVE/ScalarE.
"""

import os
import sys

sys.path.insert(0, "/opt/trn_rl_repo")

import numpy as np
import ml_dtypes

import concourse.bass as bass
import concourse.tile as tile
from concourse import bacc, mybir
from concourse.bass_utils import run_bass_kernel_spmd

F32 = mybir.dt.float32
BF16 = mybir.dt.bfloat16
AF = mybir.ActivationFunctionType
ALU = mybir.AluOpType

P = 128
N = 4096          # 64*64
H = W = 64
PAD = 8
WP = 80           # padded width/height
NPAD = WP * WP    # 6400
DILS = (1, 2, 4, 8)
NQ = 512          # positions per attention step (8 image rows)
QROWS = NQ // W   # image rows per attention slice
NSTEP = N // NQ   # 8 steps per branch
EPITCH = 9 * NQ   # Edall per-partition elements

# which tap-row (di) groups of the QK / AV products run on GpSimd
QK_POOL_DI = (1,)
AV_POOL_DI = ()

_CACHE = {}


def _sub(ap, off_elems, dims):
    """Manual free-dim sub-AP of a tile AP (keeps partition dim)."""
    part = list(ap.ap[0])
    return bass.AP(
        tensor=ap.tensor,
        offset=ap.offset + off_elems,
        ap=[part] + [[s, n] for s, n in dims],
    )


def _build():
    nc = bacc.Bacc("TRN2", target_bir_lowering=False, debug=False, num_devices=8)

    x_d = nc.dram_tensor("x", [512, N], BF16, kind="ExternalInput").ap()
    wq_d = nc.dram_tensor("wqkvT", [512, 1536], BF16, kind="ExternalInput").ap()
    wg_d = nc.dram_tensor("wgT", [512, 4], BF16, kind="ExternalInput").ap()
    bg_d = nc.dram_tensor("bg", [4, 1], F32, kind="ExternalInput").ap()
    bd_d = nc.dram_tensor("blkdiag", [128, 128], BF16, kind="ExternalInput").ap()
    sh_d = nc.dram_tensor("selhead", [18, 128], BF16, kind="ExternalInput").ap()
    id_d = nc.dram_tensor("ident", [128, 128], BF16, kind="ExternalInput").ap()
    out_d = nc.dram_tensor("out", [512, N], F32, kind="ExternalOutput").ap()

    from contextlib import ExitStack

    with tile.TileContext(nc) as tc, ExitStack() as ctx:
        consts = ctx.enter_context(tc.tile_pool(name="consts", bufs=1))
        xpool = ctx.enter_context(tc.tile_pool(name="xp", bufs=1))
        qpool = ctx.enter_context(tc.tile_pool(name="qp", bufs=2))
        kpool = ctx.enter_context(tc.tile_pool(name="kp", bufs=2))
        vpool = ctx.enter_context(tc.tile_pool(name="vp", bufs=2))
        # PSUM: pair-tiles (2 banks x2) for logits, 1-bank x2 for AV/D,
        # 1-bank x2 for gemm chunks = 8 banks total
        ldps = ctx.enter_context(tc.tile_pool(name="ldps", bufs=2, space="PSUM"))
        sps = ctx.enter_context(tc.tile_pool(name="sps", bufs=2, space="PSUM"))
        gps = ctx.enter_context(tc.tile_pool(name="gps", bufs=2, space="PSUM"))
        epool = ctx.enter_context(tc.tile_pool(name="ep", bufs=2))
        ecpool = ctx.enter_context(tc.tile_pool(name="ecp", bufs=1))
        prodp = ctx.enter_context(tc.tile_pool(name="prodp", bufs=2))
        pvp = ctx.enter_context(tc.tile_pool(name="pvp", bufs=1))
        rbpool = ctx.enter_context(tc.tile_pool(name="rbp", bufs=1))
        opool = ctx.enter_context(tc.tile_pool(name="op", bufs=2))
        gate_pool = ctx.enter_context(tc.tile_pool(name="gatep", bufs=1))

        # ---- load constants / inputs to SBUF ----
        w_sb = consts.tile([P, 4, 1536], BF16)
        for ct in range(4):
            nc.sync.dma_start(w_sb[:, ct, :], wq_d[ct * P:(ct + 1) * P, :])
        wg_sb = consts.tile([P, 4, 4], BF16)
        for ct in range(4):
            nc.sync.dma_start(wg_sb[:, ct, :], wg_d[ct * P:(ct + 1) * P, :])
        bd_sb = consts.tile([P, P], BF16)
        nc.sync.dma_start(bd_sb[:], bd_d[:])
        sh_sb = consts.tile([18, P], BF16)
        nc.sync.dma_start(sh_sb[:], sh_d[:])
        id_sb = consts.tile([P, P], BF16)
        nc.sync.dma_start(id_sb[:], id_d[:])
        bg_sb = gate_pool.tile([4, 1], F32)
        nc.sync.dma_start(bg_sb[:], bg_d[:])

        x_sb = []
        for ct in range(4):
            xt = xpool.tile([P, N], BF16, tag=f"x{ct}", name=f"x{ct}")
            x_sb.append(xt)
        qn = N // 4
        for qtr in range(4):
            for ct in range(4):
                nc.sync.dma_start(
                    x_sb[ct][:, qtr * qn:(qtr + 1) * qn],
                    x_d[ct * P:(ct + 1) * P, qtr * qn:(qtr + 1) * qn])

        # ---- gate: logits = (sum_n w_g^T x)/N + b_gate ; softmax over 4 ----
        gl_parts = gate_pool.tile([4, 8], F32)
        trash = gate_pool.tile([4, 512], F32)
        for ch in range(8):
            ps4 = sps.tile([4, 512], F32, tag="s", name=f"gps{ch}")
            for ct in range(4):
                nc.tensor.matmul(ps4[:], wg_sb[:, ct, :],
                                 x_sb[ct][:, ch * 512:(ch + 1) * 512],
                                 start=(ct == 0), stop=(ct == 3))
            nc.scalar.activation(trash[:], ps4[:], AF.Copy,
                                 accum_out=gl_parts[:, ch:ch + 1])
        glog4 = gate_pool.tile([4, 1], F32)
        nc.vector.reduce_sum(glog4[:], gl_parts[:], axis=mybir.AxisListType.X)
        logit4 = gate_pool.tile([4, 1], F32)
        nc.vector.scalar_tensor_tensor(logit4[:], glog4[:], 1.0 / N, bg_sb[:],
                                       op0=ALU.mult, op1=ALU.add)
        gexp4 = gate_pool.tile([4, 1], F32)
        nc.scalar.activation(gexp4[:], logit4[:], AF.Exp)
        gexpT = gate_pool.tile([1, 4], F32)
        nc.sync.dma_start(gexpT[:], gexp4[:])
        gsum = gate_pool.tile([1, 1], F32)
        nc.vector.reduce_sum(gsum[:], gexpT[:], axis=mybir.AxisListType.X)
        grec = gate_pool.tile([1, 1], F32)
        nc.vector.reciprocal_approx_fast(grec[:], gsum[:])
        gates = gate_pool.tile([1, 4], F32)
        nc.vector.tensor_scalar_mul(gates[:], gexpT[:], grec[:])
        gtmp = nc.dram_tensor("gtmp", [1, 4], F32).ap()
        nc.sync.dma_start(gtmp[:], gates[:])
        gate_bc = []
        for b in range(4):
            gb = gate_pool.tile([P, 1], F32, tag=f"gbc{b}")
            src = gtmp[0:1, b:b + 1]
            bc = bass.AP(tensor=src.tensor, offset=src.offset,
                         ap=[[0, P], [1, 1]])
            nc.sync.dma_start(gb[:], bc)
            gate_bc.append(gb)

        # exp grouping: single tap first so the PSUM pair-slot ring never
        # makes a late matmul wait on a pending exp
        PAIRS = [(8,), (0, 1), (2, 3), (4, 5), (6, 7)]

        def make_tiles(b):
            Q = qpool.tile([P, N], BF16, tag="Q", name=f"Q{b}")
            Kp = kpool.tile([P, NPAD], BF16, tag=f"Kp{b % 2}", bufs=1,
                            name=f"Kp{b}")
            Vp = vpool.tile([P, NPAD], BF16, tag=f"Vp{b % 2}", bufs=1,
                            name=f"Vp{b}")
            if b < 2:
                # zero borders once; interiors are fully overwritten each use
                nc.gpsimd.memset(Kp[:], 0.0)
                nc.gpsimd.memset(Vp[:], 0.0)
            return Q, Kp, Vp

        def emit_gemm_chunk(b, kind, ch, tiles):
            Q, Kp, Vp = tiles
            obase = {"q": b * P, "k": 512 + b * P, "v": 1024 + b * P}[kind]
            ps = gps.tile([P, 512], F32, tag="g", name=f"g_{b}_{kind}_{ch}")
            for ct in range(4):
                nc.tensor.matmul(
                    ps[:], w_sb[:, ct, obase:obase + P],
                    x_sb[ct][:, ch * 512:(ch + 1) * 512],
                    start=(ct == 0), stop=(ct == 3))
            if kind == "q":
                nc.scalar.activation(Q[:, ch * 512:(ch + 1) * 512],
                                     ps[:], AF.Copy)
            else:
                dst_t = Kp if kind == "k" else Vp
                dst = _sub(dst_t[:], (PAD + ch * QROWS) * WP + PAD,
                           [[WP, QROWS], [1, W]])
                ps_v = ps[:].rearrange("p (r c) -> p r c", c=W)
                if kind == "k":
                    nc.scalar.activation(dst, ps_v, AF.Copy)
                else:
                    # V evac on DVE to offload ScalarE
                    nc.vector.tensor_scalar_mul(dst, ps_v, 1.0)

        def emit_prod(b, dil, s, Q, Kp):
            """Stage A: fused QK products for step s (one step ahead)."""
            r0 = s * QROWS
            prod = prodp.tile([P, 9, NQ], BF16, tag="prod",
                              name=f"prod_{b}_{s}")
            for di in range(3):
                eng = nc.gpsimd if di in QK_POOL_DI else nc.vector
                koff = (PAD + r0 + (di - 1) * dil) * WP + (PAD - dil)
                in1 = _sub(Kp[:], koff, [[dil, 3], [WP, QROWS], [1, W]])
                in0 = _sub(Q[:], r0 * W, [[0, 3], [W, QROWS], [1, W]])
                pout = _sub(prod[:], 3 * di * NQ, [[NQ, 3], [W, QROWS], [1, W]])
                eng.tensor_mul(pout, in0, in1)
            return prod

        def emit_back(st):
            """Stage C: AV accumulate, denominator, normalize, store."""
            if st is None:
                return
            b, dil, s, Edall, Ec, Vp = st
            r0 = s * QROWS
            pv = pvp.tile([P, 9, NQ], BF16, tag="pv", name=f"pv_{b}_{s}")
            AV = sps.tile([P, NQ], F32, tag="s", name=f"AV_{b}_{s}")
            for di in range(3):
                eng = nc.gpsimd if di in AV_POOL_DI else nc.vector
                voff = (PAD + r0 + (di - 1) * dil) * WP + (PAD - dil)
                vin1 = _sub(Vp[:], voff, [[dil, 3], [WP, QROWS], [1, W]])
                ein0 = _sub(Edall[:], 3 * di * NQ, [[NQ, 3], [W, QROWS], [1, W]])
                pvout = _sub(pv[:], 3 * di * NQ, [[NQ, 3], [W, QROWS], [1, W]])
                eng.tensor_mul(pvout, ein0, vin1)
                for t in range(3 * di, 3 * di + 3):
                    nc.tensor.matmul(AV[:], id_sb[:],
                                     _sub(pv[:], t * NQ, [[1, NQ]]),
                                     start=(t == 0), stop=(t == 8))
            D = sps.tile([P, NQ], F32, tag="s", name=f"D_{b}_{s}")
            nc.tensor.matmul(D[:], sh_sb[:], Ec[:], start=True, stop=True)
            Rb = rbpool.tile([P, NQ], F32, tag="rb", name=f"Rb_{b}_{s}")
            nc.vector.reciprocal_approx_fast(Rb[:], D[:])
            osb = opool.tile([P, NQ], F32, tag="osb", name=f"osb_{b}_{s}")
            nc.vector.scalar_tensor_tensor(osb[:], AV[:], gate_bc[b][:],
                                           Rb[:], op0=ALU.mult, op1=ALU.mult)
            (nc.sync if s % 2 == 0 else nc.gpsimd).dma_start(
                out_d[b * P:(b + 1) * P, s * NQ:(s + 1) * NQ], osb[:])

        # ---- software pipeline: GEMM of branch b+1 interleaved into the
        # attention steps of branch b (3 chunks per step) ----
        CHUNKS = [(k, ch) for k in ("k", "q", "v") for ch in range(8)]
        tiles = {0: make_tiles(0)}
        for kind, ch in CHUNKS:
            emit_gemm_chunk(0, kind, ch, tiles[0])

        prev = None          # pending stage-C state
        for b, dil in enumerate(DILS):
            Q, Kp, Vp = tiles[b]
            if b + 1 < 4:
                tiles[b + 1] = make_tiles(b + 1)
                next_chunks = list(CHUNKS)
            else:
                next_chunks = []
            prev_prod = emit_prod(b, dil, 0, Q, Kp)

            for s in range(NSTEP):
                prod = prev_prod
                Edall = epool.tile([P, 9, NQ], BF16, tag="Edall",
                                   name=f"Edall_{b}_{s}")
                for pi, pair in enumerate(PAIRS):
                    npair = len(pair)
                    Ld = ldps.tile([P, npair, NQ], F32, tag="ld",
                                   name=f"Ld_{b}_{s}_{pi}")
                    for j, t in enumerate(pair):
                        nc.tensor.matmul(Ld[:, j, :], bd_sb[:],
                                         _sub(prod[:], t * NQ, [[1, NQ]]),
                                         start=True, stop=True)
                    nc.scalar.activation(Edall[:, pair[0]:pair[0] + npair, :],
                                         Ld[:], AF.Exp, scale=0.125)
                    if pi == 1:
                        # stage C of previous step fills the PE gap while
                        # ScalarE chews through the exp pairs
                        emit_back(prev)
                        # stage A: products for next step (keeps DVE ahead)
                        if s + 1 < NSTEP:
                            prev_prod = emit_prod(b, dil, s + 1, Q, Kp)

                # ---- compact denominator gather ----
                Ec = ecpool.tile([18, NQ], BF16, tag="ec", name=f"Ec_{b}_{s}")
                esrc = bass.AP(tensor=Edall.tensor, offset=Edall[:].offset,
                               ap=[[64 * EPITCH, 2], [NQ, 9], [1, NQ]])
                nc.sync.dma_start(Ec[:], esrc)
                prev = (b, dil, s, Edall, Ec, Vp)

                # next-branch GEMM chunks fill the remaining PE slack
                for _ in range(3):
                    if next_chunks:
                        kind, ch = next_chunks.pop(0)
                        emit_gemm_chunk(b + 1, kind, ch, tiles[b + 1])

        emit_back(prev)

    nc.compile()
    return nc


def _consts():
    bf = ml_dtypes.bfloat16
    blkdiag = np.zeros((128, 128), np.float32)
    blkdiag[:64, :64] = 1.0
    blkdiag[64:, 64:] = 1.0
    selhead = np.zeros((18, 128), np.float32)
    selhead[:9, :64] = 1.0
    selhead[9:, 64:] = 1.0
    ident = np.eye(128, dtype=np.float32)
    return (blkdiag.astype(bf), selhead.astype(bf), ident.astype(bf))


def kernel(x, w_qkv, w_gate, b_gate):
    x = np.asarray(x, dtype=np.float32)
    w_qkv = np.asarray(w_qkv, dtype=np.float32)
    w_gate = np.asarray(w_gate, dtype=np.float32)
    b_gate = np.asarray(b_gate, dtype=np.float32)
    if "nc" not in _CACHE:
        _CACHE["nc"] = _build()
    nc = _CACHE["nc"]
    blkdiag, selhead, ident = _consts()
    wqkvT = np.ascontiguousarray(w_qkv.T).astype(ml_dtypes.bfloat16)
    wgT = np.ascontiguousarray(w_gate.T).astype(ml_dtypes.bfloat16)
    bg = b_gate.reshape(4, 1).astype(np.float32)
    in_maps = []
    for b in range(8):
        in_maps.append({
            "x": np.ascontiguousarray(x[b].reshape(512, N)).astype(ml_dtypes.bfloat16),
            "wqkvT": wqkvT, "wgT": wgT, "bg": bg,
            "blkdiag": blkdiag, "selhead": selhead, "ident": ident,
        })
    res = run_bass_kernel_spmd(nc, in_maps, core_ids=list(range(8)),
                               trace=bool(int(os.environ.get("KTRACE", "0"))))
    _CACHE["last"] = res
    out = np.stack([np.asarray(res.results[b]["out"], dtype=np.float32)
                    .reshape(512, H, W) for b in range(8)])
    return out
